# revision 1
# baseline (speedup 1.0000x reference)
"""Trainium2 Bass kernel for nn_Lowpass: 2D DCT -> keep 15x15 low-freq block -> 2D IDCT.

The whole op collapses to out[b,c] = P @ x[b,c] @ P^T with P = Di[:, :15] @ D[:15, :]
(a fixed 32x32 projection). Per 32x32 image that is two 32x32 matmuls, which map
onto the PE array as 16-way tile_position-packed matmuls (K=M=32) with the
constant P^T stationary, streaming 512-wide slabs of 16 images, with DVE 32x32
block transposes (which double as the PSUM->SBUF eviction) between/after the two
matmul rounds. Data parallel across 8 NeuronCores: 3072 images per core.
"""

import numpy as np

N = 32
FRE = 15
NCORES = 8
IMG_TOTAL = 8192 * 3          # 24576 images of 32x32
PER_CORE = IMG_TOTAL // NCORES  # 3072
PACK = 256                    # images per pipeline iteration (1 MB)
NPACK = PER_CORE // PACK      # 12


def _install_tilefix():
    """This container's walrus build rejects instructions carrying >1 sem wait
    ("Too many sync wait commands" in setupSyncWait). Tile attaches all of an
    instruction's required waits to the instruction itself. Split: for any
    instruction with N>1 waits, hoist N-1 of them onto fresh same-engine nop
    instructions placed immediately before it (same blocking semantics, one
    wait per instruction). Same treatment for the kernel-tail drain."""
    from concourse import mybir, tile
    from concourse.vector_clock import ScopedClock, VectorClock

    if getattr(tile.TileContext, "_tilefix_installed", False):
        return

    orig_lower = tile.TileContext._lower_ordered_insts

    def _lower_split(self, postordered_blocks):
        nc = self.nc
        for insts in postordered_blocks.values():
            new = []
            for inst in insts:
                si = getattr(inst, "sync_info", None)
                ow = list(si.on_wait) if si is not None and si.on_wait else []
                if len(ow) > 1:
                    for w in ow[:-1]:
                        nop = mybir.InstNoOp(
                            name=nc.get_next_instruction_name(), ins=[], outs=[])
                        nop.engine = inst.engine
                        nop.sync_info = mybir.SyncInfo(
                            on_wait=[w], on_update=[])
                        new.append(nop)
                    inst.sync_info = mybir.SyncInfo(
                        on_wait=[ow[-1]], on_update=list(si.on_update))
                new.append(inst)
            insts[:] = new
        return orig_lower(self, postordered_blocks)

    def _drain_and_barrier_split(self, tick_clock, wait_clock):
        nc = self.nc
        gc = tick_clock.global_clock
        n = len(gc)
        for proc in range(n):
            t = gc[proc]
            if t <= 0:
                continue
            vec = [0] * n
            vec[proc] = t
            nop_inst = nc.sync.nop()
            wait_clock.add_sem_waits(
                nop_inst.ins, ScopedClock({None: VectorClock(vec)})
            )
        nc.sync.drain()
        nc.all_engine_barrier()
        assert self.sems is not None
        popped = nc._tile_sem_poison_stack.pop()
        assert popped is self._sem_poison
        nc.clear_and_free_semaphores(list(self.sems.allocated().values()))
        nc.all_engine_barrier()

    tile.TileContext._lower_ordered_insts = _lower_split
    tile.TileContext._drain_and_barrier = _drain_and_barrier_split
    tile.TileContext._tilefix_installed = True

    # NTFF profiling hooks don't exist in this container; make trace=True
    # degrade gracefully inside run_bass_kernel_spmd.
    import sys as _sys
    import types as _types
    if "antenv.axon_hooks" not in _sys.modules:
        m = _types.ModuleType("antenv.axon_hooks")
        m.get_axon_ntff_profile_hook = lambda: None
        _sys.modules["antenv.axon_hooks"] = m


def _p_matrix():
    i = np.arange(N)
    D = 2.0 * np.cos(np.pi * (2 * i[None, :] + 1) * i[:, None] / (2 * N))
    Di = np.linalg.inv(D)
    P = Di[:, :FRE] @ D[:FRE, :]        # float64 [32, 32]
    return P


def _build_program(mm_dtype_name="float32", loop_reps=1, dma_only=False):
    from concourse import bass, tile
    from concourse import mybir

    F32 = mybir.dt.float32
    MMDT = getattr(mybir.dt, mm_dtype_name)

    nc = bass.Bass("TRN2", target_bir_lowering=False, debug=False,
                   num_devices=NCORES)
    x_ext = nc.dram_tensor("x", [PER_CORE, N, N], F32, kind="ExternalInput").ap()
    p_ext = nc.dram_tensor("pconst", [128, N], F32, kind="ExternalInput").ap()
    y_ext = nc.dram_tensor("y", [PER_CORE, N, N], F32, kind="ExternalOutput").ap()

    with tile.TileContext(nc) as tc:
        with tc.tile_pool(name="const", bufs=1) as cpool, \
             tc.tile_pool(name="xin", bufs=3) as xpool, \
             tc.tile_pool(name="tmid", bufs=2) as tpool, \
             tc.tile_pool(name="yout", bufs=2) as ypool, \
             tc.tile_pool(name="psA", bufs=1, space="PSUM") as papool, \
             tc.tile_pool(name="psB", bufs=1, space="PSUM") as pbpool:

            pc = cpool.tile([128, N], F32)
            nc.sync.dma_start(pc[:], p_ext[:])
            pc_mm = pc.bitcast(MMDT)

            for p_rep in range(NPACK * loop_reps):
                p = p_rep % NPACK
                base = p * PACK
                # ---- load: ONE 1MB DMA per pack ----
                # X[32r+h, 32*ci+w] = x[base + 4*ci + r][h, w]; (r,h) merges
                # into the 128-partition dim, ci is one uniform-stride dim.
                X = xpool.tile([128, 2048], F32)
                src = x_ext[base: base + PACK]
                nc.sync.dma_start(
                    X.rearrange("p (ci w) -> p ci w", w=N),
                    src.rearrange("(ci r) h w -> r h ci w", r=4),
                )
                X_mm = X.bitcast(MMDT)

                if dma_only:
                    dst0 = y_ext[base: base + PACK]
                    nc.scalar.dma_start(
                        dst0.rearrange("(ci r) h w -> r h ci w", r=4),
                        X.rearrange("p (ci w) -> p ci w", w=N),
                    )
                    continue

                # ---- round 1: t_n = P @ x_n  (16 packed matmuls) ----
                # tile (r, c): out[u, 32i+w] = sum_h PT[h,u] x_n[h,w]
                # psum bank r = pa[:, 512r:512(r+1)]
                T = tpool.tile([128, 2048], F32)
                pa = papool.tile([128, 2048], F32, tag="psA")
                for r in range(4):
                    for c in range(4):
                        nc.tensor.matmul(
                            pa[32 * c:32 * (c + 1), 512 * r:512 * (r + 1)],
                            pc_mm[32 * r:32 * (r + 1), :],
                            X_mm[32 * r:32 * (r + 1), 512 * c:512 * (c + 1)],
                            start=True, stop=True,
                            tile_position=(32 * r, 32 * c),
                        )
                # blockwise 32x32 transpose, also evicts PSUM -> SBUF
                nc.vector.transpose(T[:], pa[:])
                T_mm = T.bitcast(MMDT)

                # ---- round 2: y_n = t_n @ P^T (16 packed matmuls) ----
                # tile (r, c): rhs = t^T blocks, out = y^T blocks
                # psum bank c = pb[:, 512c:512(c+1)]
                Y = ypool.tile([128, 2048], F32)
                pb = pbpool.tile([128, 2048], F32, tag="psB")
                for c in range(4):
                    for r in range(4):
                        nc.tensor.matmul(
                            pb[32 * r:32 * (r + 1), 512 * c:512 * (c + 1)],
                            pc_mm[32 * c:32 * (c + 1), :],
                            T_mm[32 * c:32 * (c + 1), 512 * r:512 * (r + 1)],
                            start=True, stop=True,
                            tile_position=(32 * c, 32 * r),
                        )
                nc.vector.transpose(Y[:], pb[:])

                # ---- store: ONE 1MB DMA per pack ----
                # Y[32r+h, 32*ci+w] = y[base + 4*ci + r][h, w]
                dst = y_ext[base: base + PACK]
                nc.scalar.dma_start(
                    dst.rearrange("(ci r) h w -> r h ci w", r=4),
                    Y.rearrange("p (ci w) -> p ci w", w=N),
                )

    return nc


def _run(x_flat, trace=False, mm_dtype_name="float32"):
    from concourse.bass_utils import run_bass_kernel_spmd

    _install_tilefix()
    nc = _build_program(mm_dtype_name)

    P = _p_matrix()
    PT = np.ascontiguousarray(P.T.astype(np.float32))   # PT[h, u] = P[u, h]
    pconst = np.tile(PT, (4, 1))                        # [128, 32]

    core_ids = list(range(NCORES))
    in_maps = [
        {"x": np.ascontiguousarray(x_flat[i * PER_CORE:(i + 1) * PER_CORE]),
         "pconst": pconst}
        for i in core_ids
    ]
    bkr = run_bass_kernel_spmd(nc, in_maps, core_ids, trace=trace)
    out = np.concatenate([bkr.results[i]["y"] for i in core_ids], axis=0)
    return out, bkr


def kernel(x):
    x = np.asarray(x, dtype=np.float32)
    x_flat = x.reshape(IMG_TOTAL, N, N)
    out, _ = _run(x_flat, trace=False)
    return out.reshape(x.shape).astype(np.float32)

